# revision 12
# baseline (speedup 1.0000x reference)
"""Single-head attention layer (B=4, S=2048, D=H=1024) on 8 TRN2 NeuronCores.

Sharding: core c -> batch c//2, sequence-half c%2 (1024 rows per core).
Each core projects Q/K/V for its own 1024 rows only; the two cores of a
batch exchange K/V halves with 2-core AllGathers, so no projection work
is duplicated. Scores are computed transposed so softmax needs no
on-chip transposes and no max-subtraction (|scores*scale| < ~3 here).

All matmuls run in bf16 with fp32 PSUM accumulation:
  QT[h,q]    = matmul(lhsT=Wq[d,h], rhs=xq[d,q])     (+bq via ACT bias)
  Kh[s,h]    = matmul(lhsT=xq[d,s], rhs=Wk[d,h])     (+bk via DVE bcast add)
  Vh[s,h]    = matmul(lhsT=xq[d,s], rhs=Wv[d,h])     (+bv via DVE bcast add)
  K,V        = AllGather(Kh), AllGather(Vh) over pairs {2b, 2b+1}
  KT[h,k]    <- DMA-transpose load of K
  ST[k,q]    = matmul(lhsT=KT[h,k], rhs=QT[h,q])
  ET[k,q]    = exp(ST * 1/sqrt(H))
  O[q,h]     = matmul(lhsT=ET[k,q], rhs=V[k,h])      (accumulate over k)
  den[q,1]   = matmul(lhsT=ET[k,q], rhs=ones[k,1])
  out        = O * (1/den)
"""

import os

import numpy as np
import ml_dtypes

B, S, D, H = 4, 2048, 1024, 1024
NCORES = 8
PT = 128            # partition tile
CH = 512            # psum free-dim chunk (fp32 bank limit)
QH = S // 2         # rows per core
ND = D // PT        # 8 d-tiles
NHT = H // PT       # 8 h-tiles
NKT = S // PT       # 16 k/s-tiles (full sequence)
NST = QH // PT      # 8 s-tiles in this core's half
NQT = QH // PT      # 8 q-tiles per core
SCALE = 1.0 / float(np.sqrt(H))

BF16 = ml_dtypes.bfloat16

_NC = None


def _build():
    import concourse.bacc as bacc
    import concourse.mybir as mybir
    from concourse.tile import TileContext

    dt = mybir.dt
    AF = mybir.ActivationFunctionType
    GROUPS = [[0, 1], [2, 3], [4, 5], [6, 7]]

    nc = bacc.Bacc(None, target_bir_lowering=False, num_devices=NCORES,
                   num_swdge_queues=4)

    xq = nc.declare_dram_parameter("xq", [D, QH], dt.bfloat16, isOutput=False)
    wq = nc.declare_dram_parameter("wq", [D, H], dt.bfloat16, isOutput=False)
    wk = nc.declare_dram_parameter("wk", [D, H], dt.bfloat16, isOutput=False)
    wv = nc.declare_dram_parameter("wv", [D, H], dt.bfloat16, isOutput=False)
    bqr = nc.declare_dram_parameter("bqr", [PT, NHT], dt.float32, isOutput=False)
    bkb = nc.declare_dram_parameter("bkb", [PT, H], dt.bfloat16, isOutput=False)
    bvb = nc.declare_dram_parameter("bvb", [PT, H], dt.bfloat16, isOutput=False)
    y = nc.declare_dram_parameter("y", [QH, H], dt.float32, isOutput=True)

    with TileContext(nc) as tc:
        with (
            tc.tile_pool(name="px", bufs=ND) as px,        # xq tiles then ET tiles
            tc.tile_pool(name="pw", bufs=3 * ND) as pw,
            tc.tile_pool(name="pqt", bufs=NHT) as pqt,
            tc.tile_pool(name="pkt", bufs=NHT) as pkt,
            tc.tile_pool(name="pv", bufs=NKT) as pv,
            tc.tile_pool(name="pmisc", bufs=1) as pmisc,
            tc.tile_pool(name="phalf", bufs=4) as phalf,
            tc.tile_pool(name="pstage", bufs=4) as pstage,
            tc.tile_pool(name="prd", bufs=2) as prd,
            tc.tile_pool(name="pdram", bufs=1, space="DRAM") as pdram,
            tc.tile_pool(name="psum", bufs=8, space="PSUM") as pp,
        ):
            # ---- DRAM bounce tensors for the K/V exchange ----
            # K is exchanged in two pipelined chunks so scores can start on
            # chunk 0 while chunk 1 is still in flight.
            kh_d = [pdram.tile([QH // 2, H], dt.bfloat16, tag=f"khd{i}", name="khd")
                    for i in range(2)]
            kf_d = [pdram.tile([QH, H], dt.bfloat16, tag=f"kfd{i}", name="kfd")
                    for i in range(2)]
            vh_d = pdram.tile([QH, H], dt.bfloat16, tag="vhd")
            vf_d = pdram.tile([S, H], dt.bfloat16, tag="vfd")

            # ---- loads (xq/wk interleaved: phase A1 consumes them d-major) ----
            xq_t = []
            w_t = {}
            for d in range(ND):
                t = px.tile([PT, 2 * QH], dt.bfloat16, tag="xt", name="xt")
                nc.sync.dma_start(out=t[:, 0:QH], in_=xq[d * PT:(d + 1) * PT, :])
                xq_t.append(t)
                tw = pw.tile([PT, H], dt.bfloat16, tag="w", name="wt")
                nc.sync.dma_start(out=tw[:], in_=wk[d * PT:(d + 1) * PT, :])
                w_t["wk", d] = tw
            for name, hnd in (("wv", wv), ("wq", wq)):
                for d in range(ND):
                    t = pw.tile([PT, H], dt.bfloat16, tag="w", name="wt")
                    nc.sync.dma_start(out=t[:], in_=hnd[d * PT:(d + 1) * PT, :])
                    w_t[name, d] = t
            bq_t = pmisc.tile([PT, NHT], dt.float32, tag="bq")
            nc.sync.dma_start(out=bq_t[:], in_=bqr[:, :])
            bk_t = pmisc.tile([PT, H], dt.bfloat16, tag="bk")
            nc.sync.dma_start(out=bk_t[:], in_=bkb[:, :])
            bv_t = pmisc.tile([PT, H], dt.bfloat16, tag="bv")
            nc.sync.dma_start(out=bv_t[:], in_=bvb[:, :])
            ones_t = pmisc.tile([PT, 1], dt.bfloat16, tag="ones")
            nc.vector.memset(ones_t[:], 1.0)

            # ---- phase A1: K-half projection in two chunks, each exported
            # and gathered as soon as it is done. d-major order with all 8
            # PSUM groups of a chunk live, so the first matmul only needs
            # xq[0]/wk[0] to have landed.
            for sb in range(2):
                sts = range(sb * NST // 2, (sb + 1) * NST // 2)
                ps = {(st, hc): pp.tile([PT, CH], dt.float32, tag="big", name="psb")
                      for st in sts for hc in range(2)}
                for d in range(ND):
                    for st in sts:
                        lhs = xq_t[d][:, st * PT:(st + 1) * PT]
                        for hc in range(2):
                            nc.tensor.matmul(
                                ps[st, hc][:], lhs,
                                w_t["wk", d][:, hc * CH:(hc + 1) * CH],
                                start=(d == 0), stop=(d == ND - 1),
                            )
                for st in sts:
                    half = phalf.tile([PT, H], dt.bfloat16, tag="half", name="half")
                    for hc in range(2):
                        nc.vector.tensor_add(
                            half[:, hc * CH:(hc + 1) * CH], ps[st, hc][:],
                            bk_t[:, hc * CH:(hc + 1) * CH],
                        )
                    nc.gpsimd.dma_start(
                        out=kh_d[sb][(st % 4) * PT:(st % 4 + 1) * PT, :], in_=half[:],
                    )
                nc.gpsimd.collective_compute(
                    "AllGather", mybir.AluOpType.bypass, replica_groups=GROUPS,
                    ins=[kh_d[sb][:]], outs=[kf_d[sb][:]],
                )

            # ---- V-half projection + single gather ----
            for st in range(NST):
                ps = [pp.tile([PT, CH], dt.float32, tag="big", name="psb")
                      for _ in range(2)]
                for d in range(ND):
                    lhs = xq_t[d][:, st * PT:(st + 1) * PT]
                    for hc in range(2):
                        nc.tensor.matmul(
                            ps[hc][:], lhs,
                            w_t["wv", d][:, hc * CH:(hc + 1) * CH],
                            start=(d == 0), stop=(d == ND - 1),
                        )
                half = phalf.tile([PT, H], dt.bfloat16, tag="half", name="half")
                for hc in range(2):
                    nc.vector.tensor_add(
                        half[:, hc * CH:(hc + 1) * CH], ps[hc][:],
                        bv_t[:, hc * CH:(hc + 1) * CH],
                    )
                nc.gpsimd.dma_start(
                    out=vh_d[st * PT:(st + 1) * PT, :], in_=half[:],
                )
            nc.gpsimd.collective_compute(
                "AllGather", mybir.AluOpType.bypass, replica_groups=GROUPS,
                ins=[vh_d[:]], outs=[vf_d[:]],
            )

            # ---- phase A2: Q projection (fills PE while collectives run) ----
            qt_t = []
            for h in range(NHT):
                qtile = pqt.tile([PT, QH], dt.bfloat16, tag="qt")
                qt_t.append(qtile)
                ps = [pp.tile([PT, CH], dt.float32, tag="big", name="psb")
                      for _ in range(2)]
                for d in range(ND):
                    lhs = w_t["wq", d][:, h * PT:(h + 1) * PT]
                    for c in range(2):
                        nc.tensor.matmul(
                            ps[c][:], lhs, xq_t[d][:, c * CH:(c + 1) * CH],
                            start=(d == 0), stop=(d == ND - 1),
                        )
                for c in range(2):
                    nc.scalar.activation(
                        qtile[:, c * CH:(c + 1) * CH], ps[c][:],
                        AF.Identity, bias=bq_t[:, h:h + 1],
                    )

            # ---- gather loads: KT via DMA-transpose (chunk-major), V natural ----
            # kf_d[sb] rows [0:512] hold global k-tiles 4sb..4sb+3 (rank 0),
            # rows [512:1024] hold global k-tiles 8+4sb..8+4sb+3 (rank 1).
            kt_t = [pkt.tile([PT, S], dt.bfloat16, tag="kt", name="ktile")
                    for _ in range(NHT)]
            for sb in range(2):
                for h in range(NHT):
                    nc.sync.dma_start(
                        out=kt_t[h][:, sb * 4 * PT:(sb * 4 + 4) * PT],
                        in_=kf_d[sb][0:QH // 2, h * PT:(h + 1) * PT],
                        transpose=True,
                    )
                    nc.sync.dma_start(
                        out=kt_t[h][:, (8 + sb * 4) * PT:(8 + sb * 4 + 4) * PT],
                        in_=kf_d[sb][QH // 2:QH, h * PT:(h + 1) * PT],
                        transpose=True,
                    )
            v_t = []
            for st in range(NKT):
                vtile = pv.tile([PT, H], dt.bfloat16, tag="v")
                v_t.append(vtile)
                nc.sync.dma_start(
                    out=vtile[:], in_=vf_d[st * PT:(st + 1) * PT, :],
                )

            # ---- phase B: scores^T + exp ----
            # ET stored as 8 tiles [PT, 2*QH] (two k-tiles each), reusing
            # the xq pool slots (tag "xt").
            et_t = []
            for i in range(ND):
                et_t.append(px.tile([PT, 2 * QH], dt.bfloat16, tag="xt", name="et"))

            def et_slice(kt, q0, qn):
                return et_t[kt // 2][:, (kt % 2) * QH + q0:(kt % 2) * QH + q0 + qn]

            KT_ORDER = [0, 1, 2, 3, 8, 9, 10, 11, 4, 5, 6, 7, 12, 13, 14, 15]
            for kt in KT_ORDER:
                ps = [pp.tile([PT, CH], dt.float32, tag="big", name="psb")
                      for _ in range(2)]
                for h in range(NHT):
                    lhs = kt_t[h][:, kt * PT:(kt + 1) * PT]
                    for qc in range(2):
                        nc.tensor.matmul(
                            ps[qc][:], lhs, qt_t[h][:, qc * CH:(qc + 1) * CH],
                            start=(h == 0), stop=(h == NHT - 1),
                        )
                for qc in range(2):
                    nc.scalar.activation(
                        et_slice(kt, qc * CH, CH), ps[qc][:], AF.Exp, scale=SCALE,
                    )

            # ---- phase C: attn @ V, denominator, normalize ----
            for qt in range(NQT):
                dn = pp.tile([PT, 1], dt.float32, tag="big", name="dn")
                po = [pp.tile([PT, CH], dt.float32, tag="big", name="psb")
                      for _ in range(2)]
                for kt in range(NKT):
                    lhs = et_slice(kt, qt * PT, PT)
                    for hc in range(2):
                        nc.tensor.matmul(
                            po[hc][:], lhs, v_t[kt][:, hc * CH:(hc + 1) * CH],
                            start=(kt == 0), stop=(kt == NKT - 1),
                        )
                    nc.tensor.matmul(
                        dn[:], lhs, ones_t[:, 0:1],
                        start=(kt == 0), stop=(kt == NKT - 1),
                    )
                rd = prd.tile([PT, 1], dt.float32, tag="rd")
                nc.vector.reciprocal(rd[:], dn[:])
                for hc in range(2):
                    stage = pstage.tile([PT, CH], dt.float32, tag="st", name="stage")
                    nc.vector.tensor_scalar_mul(stage[:], po[hc][:], rd[:])
                    nc.sync.dma_start(
                        out=y[qt * PT:(qt + 1) * PT, hc * CH:(hc + 1) * CH],
                        in_=stage[:],
                    )

    return nc


def _get_nc():
    global _NC
    if _NC is None:
        nc = _build()
        nc.finalize()
        _NC = nc
    return _NC


def kernel(x, Wq, bq, Wk, bk, Wv, bv):
    from concourse.bass_utils import run_bass_kernel_spmd

    nc = _get_nc()

    wq_b = np.ascontiguousarray(Wq.astype(BF16))
    wk_b = np.ascontiguousarray(Wk.astype(BF16))
    wv_b = np.ascontiguousarray(Wv.astype(BF16))
    bq_r = np.ascontiguousarray(bq.reshape(NHT, PT).T.astype(np.float32))
    bk_b = np.ascontiguousarray(np.broadcast_to(bk.astype(BF16), (PT, H)))
    bv_b = np.ascontiguousarray(np.broadcast_to(bv.astype(BF16), (PT, H)))

    in_maps = []
    for c in range(NCORES):
        b, qh = divmod(c, 2)
        xq_c = np.ascontiguousarray(x[b, qh * QH:(qh + 1) * QH, :].T.astype(BF16))
        in_maps.append({
            "xq": xq_c,
            "wq": wq_b, "wk": wk_b, "wv": wv_b,
            "bqr": bq_r, "bkb": bk_b, "bvb": bv_b,
        })

    trace = bool(os.environ.get("BASS_KERNEL_TRACE"))
    kwargs = {}
    if trace:
        _register_ntff_hook()
        kwargs = {"trace": True, "tmpdir": os.environ.get("BASS_KERNEL_TRACE_DIR")}

    res = run_bass_kernel_spmd(nc, in_maps, list(range(NCORES)), **kwargs)
    if trace:
        kernel.last_exec_time_ns = res.exec_time_ns
        kernel.last_results = res

    out = np.empty((B, S, H), np.float32)
    for c in range(NCORES):
        b, qh = divmod(c, 2)
        out[b, qh * QH:(qh + 1) * QH, :] = res.results[c]["y"]
    return out


def _register_ntff_hook():
    """The container's antenv lacks axon_hooks; register it so trace=True
    can capture NTFF profiles through the axon PJRT library."""
    import sys
    import types

    if "antenv.axon_hooks" in sys.modules:
        return
    mod = types.ModuleType("antenv.axon_hooks")
    holder = [None]
    mod.set_axon_ntff_profile_hook = lambda h: holder.__setitem__(0, h)
    mod.get_axon_ntff_profile_hook = lambda: holder[0]
    sys.modules["antenv.axon_hooks"] = mod
    import antenv

    antenv.axon_hooks = mod
    from trn_agent_boot.trn_boot import _ntff_profile_via_ctypes

    mod.set_axon_ntff_profile_hook(_ntff_profile_via_ctypes("/opt/axon/libaxon_pjrt.so"))


# revision 13
# speedup vs baseline: 1.1253x; 1.1253x over previous
"""Single-head attention layer (B=4, S=2048, D=H=1024) on 8 TRN2 NeuronCores.

Sharding: core c -> batch c//2, sequence-half c%2 (1024 rows per core).
Each core projects Q/K/V for its own 1024 rows only; the two cores of a
batch exchange K/V halves with 2-core AllGathers, so no projection work
is duplicated. Scores are computed transposed so softmax needs no
on-chip transposes and no max-subtraction (|scores*scale| < ~3 here).

All matmuls run in bf16 with fp32 PSUM accumulation:
  QT[h,q]    = matmul(lhsT=Wq[d,h], rhs=xq[d,q])     (+bq via ACT bias)
  Kh[s,h]    = matmul(lhsT=xq[d,s], rhs=Wk[d,h])     (+bk via DVE bcast add)
  Vh[s,h]    = matmul(lhsT=xq[d,s], rhs=Wv[d,h])     (+bv via DVE bcast add)
  K,V        = AllGather(Kh), AllGather(Vh) over pairs {2b, 2b+1}
  KT[h,k]    <- DMA-transpose load of K
  ST[k,q]    = matmul(lhsT=KT[h,k], rhs=QT[h,q])
  ET[k,q]    = exp(ST * 1/sqrt(H))
  O[q,h]     = matmul(lhsT=ET[k,q], rhs=V[k,h])      (accumulate over k)
  den[q,1]   = matmul(lhsT=ET[k,q], rhs=ones[k,1])
  out        = O * (1/den)
"""

import os

import numpy as np
import ml_dtypes

B, S, D, H = 4, 2048, 1024, 1024
NCORES = 8
PT = 128            # partition tile
CH = 512            # psum free-dim chunk (fp32 bank limit)
QH = S // 2         # rows per core
ND = D // PT        # 8 d-tiles
NHT = H // PT       # 8 h-tiles
NKT = S // PT       # 16 k/s-tiles (full sequence)
NST = QH // PT      # 8 s-tiles in this core's half
NQT = QH // PT      # 8 q-tiles per core
SCALE = 1.0 / float(np.sqrt(H))

BF16 = ml_dtypes.bfloat16

_NC = None


def _build():
    import concourse.bacc as bacc
    import concourse.mybir as mybir
    from concourse.tile import TileContext

    dt = mybir.dt
    AF = mybir.ActivationFunctionType
    GROUPS = [[0, 1], [2, 3], [4, 5], [6, 7]]

    nc = bacc.Bacc(None, target_bir_lowering=False, num_devices=NCORES,
                   num_swdge_queues=4)

    xq = nc.declare_dram_parameter("xq", [D, QH], dt.bfloat16, isOutput=False)
    wq = nc.declare_dram_parameter("wq", [D, H], dt.bfloat16, isOutput=False)
    wk = nc.declare_dram_parameter("wk", [D, H], dt.bfloat16, isOutput=False)
    wv = nc.declare_dram_parameter("wv", [D, H], dt.bfloat16, isOutput=False)
    bqr = nc.declare_dram_parameter("bqr", [PT, NHT], dt.float32, isOutput=False)
    bkb = nc.declare_dram_parameter("bkb", [PT, H], dt.bfloat16, isOutput=False)
    bvb = nc.declare_dram_parameter("bvb", [PT, H], dt.bfloat16, isOutput=False)
    y = nc.declare_dram_parameter("y", [QH, H], dt.float32, isOutput=True)

    with TileContext(nc) as tc:
        with (
            tc.tile_pool(name="px", bufs=ND) as px,        # xq tiles then ET tiles
            tc.tile_pool(name="pw", bufs=3 * ND) as pw,
            tc.tile_pool(name="pqt", bufs=NHT) as pqt,
            tc.tile_pool(name="pkt", bufs=NHT) as pkt,
            tc.tile_pool(name="pv", bufs=NKT) as pv,
            tc.tile_pool(name="pmisc", bufs=1) as pmisc,
            tc.tile_pool(name="phalf", bufs=4) as phalf,
            tc.tile_pool(name="pstage", bufs=4) as pstage,
            tc.tile_pool(name="prd", bufs=2) as prd,
            tc.tile_pool(name="pdram", bufs=1, space="DRAM") as pdram,
            tc.tile_pool(name="psum", bufs=8, space="PSUM") as pp,
        ):
            # ---- DRAM bounce tensors for the K/V exchange ----
            # K is exchanged in two pipelined chunks so scores can start on
            # chunk 0 while chunk 1 is still in flight.
            kh_d = [pdram.tile([QH // 2, H], dt.bfloat16, tag=f"khd{i}", name="khd")
                    for i in range(2)]
            kf_d = [pdram.tile([QH, H], dt.bfloat16, tag=f"kfd{i}", name="kfd")
                    for i in range(2)]
            vh_d = pdram.tile([QH, H], dt.bfloat16, tag="vhd")
            vf_d = pdram.tile([S, H], dt.bfloat16, tag="vfd")

            # ---- loads (xq/wk interleaved: phase A1 consumes them d-major) ----
            bq_t = pmisc.tile([PT, NHT], dt.float32, tag="bq")
            nc.sync.dma_start(out=bq_t[:], in_=bqr[:, :])
            bk_t = pmisc.tile([PT, H], dt.bfloat16, tag="bk")
            nc.sync.dma_start(out=bk_t[:], in_=bkb[:, :])
            bv_t = pmisc.tile([PT, H], dt.bfloat16, tag="bv")
            nc.sync.dma_start(out=bv_t[:], in_=bvb[:, :])
            ones_t = pmisc.tile([PT, 1], dt.bfloat16, tag="ones")
            nc.vector.memset(ones_t[:], 1.0)
            xq_t = []
            w_t = {}
            for d in range(ND):
                t = px.tile([PT, 2 * QH], dt.bfloat16, tag="xt", name="xt")
                nc.sync.dma_start(out=t[:, 0:QH], in_=xq[d * PT:(d + 1) * PT, :])
                xq_t.append(t)
                tw = pw.tile([PT, H], dt.bfloat16, tag="w", name="wt")
                nc.sync.dma_start(out=tw[:], in_=wk[d * PT:(d + 1) * PT, :])
                w_t["wk", d] = tw
            for name, hnd in (("wv", wv), ("wq", wq)):
                for d in range(ND):
                    t = pw.tile([PT, H], dt.bfloat16, tag="w", name="wt")
                    nc.sync.dma_start(out=t[:], in_=hnd[d * PT:(d + 1) * PT, :])
                    w_t[name, d] = t

            # ---- phase A1: K-half projection in two chunks, each exported
            # and gathered as soon as it is done. d-major order with all 8
            # PSUM groups of a chunk live, so the first matmul only needs
            # xq[0]/wk[0] to have landed.
            for sb in range(2):
                sts = range(sb * NST // 2, (sb + 1) * NST // 2)
                ps = {(st, hc): pp.tile([PT, CH], dt.float32, tag="big", name="psb")
                      for st in sts for hc in range(2)}
                for d in range(ND):
                    for st in sts:
                        lhs = xq_t[d][:, st * PT:(st + 1) * PT]
                        for hc in range(2):
                            nc.tensor.matmul(
                                ps[st, hc][:], lhs,
                                w_t["wk", d][:, hc * CH:(hc + 1) * CH],
                                start=(d == 0), stop=(d == ND - 1),
                            )
                for st in sts:
                    half = phalf.tile([PT, H], dt.bfloat16, tag="half", name="half")
                    for hc in range(2):
                        nc.vector.tensor_add(
                            half[:, hc * CH:(hc + 1) * CH], ps[st, hc][:],
                            bk_t[:, hc * CH:(hc + 1) * CH],
                        )
                    nc.gpsimd.dma_start(
                        out=kh_d[sb][(st % 4) * PT:(st % 4 + 1) * PT, :], in_=half[:],
                    )
                with tc.high_priority():
                    nc.gpsimd.collective_compute(
                        "AllGather", mybir.AluOpType.bypass, replica_groups=GROUPS,
                        ins=[kh_d[sb][:]], outs=[kf_d[sb][:]],
                    )

            # ---- V-half projection + single gather ----
            for st in range(NST):
                ps = [pp.tile([PT, CH], dt.float32, tag="big", name="psb")
                      for _ in range(2)]
                for d in range(ND):
                    lhs = xq_t[d][:, st * PT:(st + 1) * PT]
                    for hc in range(2):
                        nc.tensor.matmul(
                            ps[hc][:], lhs,
                            w_t["wv", d][:, hc * CH:(hc + 1) * CH],
                            start=(d == 0), stop=(d == ND - 1),
                        )
                half = phalf.tile([PT, H], dt.bfloat16, tag="half", name="half")
                for hc in range(2):
                    nc.vector.tensor_add(
                        half[:, hc * CH:(hc + 1) * CH], ps[hc][:],
                        bv_t[:, hc * CH:(hc + 1) * CH],
                    )
                nc.gpsimd.dma_start(
                    out=vh_d[st * PT:(st + 1) * PT, :], in_=half[:],
                )
            with tc.high_priority():
                nc.gpsimd.collective_compute(
                    "AllGather", mybir.AluOpType.bypass, replica_groups=GROUPS,
                    ins=[vh_d[:]], outs=[vf_d[:]],
                )

            # ---- phase A2: Q projection (fills PE while collectives run) ----
            qt_t = []
            for h in range(NHT):
                qtile = pqt.tile([PT, QH], dt.bfloat16, tag="qt")
                qt_t.append(qtile)
                ps = [pp.tile([PT, CH], dt.float32, tag="big", name="psb")
                      for _ in range(2)]
                for d in range(ND):
                    lhs = w_t["wq", d][:, h * PT:(h + 1) * PT]
                    for c in range(2):
                        nc.tensor.matmul(
                            ps[c][:], lhs, xq_t[d][:, c * CH:(c + 1) * CH],
                            start=(d == 0), stop=(d == ND - 1),
                        )
                for c in range(2):
                    nc.scalar.activation(
                        qtile[:, c * CH:(c + 1) * CH], ps[c][:],
                        AF.Identity, bias=bq_t[:, h:h + 1],
                    )

            # ---- gather loads: KT via DMA-transpose (chunk-major), V natural ----
            # kf_d[sb] rows [0:512] hold global k-tiles 4sb..4sb+3 (rank 0),
            # rows [512:1024] hold global k-tiles 8+4sb..8+4sb+3 (rank 1).
            kt_t = [pkt.tile([PT, S], dt.bfloat16, tag="kt", name="ktile")
                    for _ in range(NHT)]
            with tc.high_priority():
                for sb in range(2):
                    for h in range(NHT):
                        nc.sync.dma_start(
                            out=kt_t[h][:, sb * 4 * PT:(sb * 4 + 4) * PT],
                            in_=kf_d[sb][0:QH // 2, h * PT:(h + 1) * PT],
                            transpose=True,
                        )
                        nc.sync.dma_start(
                            out=kt_t[h][:, (8 + sb * 4) * PT:(8 + sb * 4 + 4) * PT],
                            in_=kf_d[sb][QH // 2:QH, h * PT:(h + 1) * PT],
                            transpose=True,
                        )

            # ---- phase B: scores^T + exp ----
            # ET stored as 8 tiles [PT, 2*QH] (two k-tiles each), reusing
            # the xq pool slots (tag "xt").
            et_t = []
            for i in range(ND):
                et_t.append(px.tile([PT, 2 * QH], dt.bfloat16, tag="xt", name="et"))

            def et_slice(kt, q0, qn):
                return et_t[kt // 2][:, (kt % 2) * QH + q0:(kt % 2) * QH + q0 + qn]

            KT_ORDER = [0, 1, 2, 3, 8, 9, 10, 11, 4, 5, 6, 7, 12, 13, 14, 15]
            for kt in KT_ORDER:
                ps = [pp.tile([PT, CH], dt.float32, tag="big", name="psb")
                      for _ in range(2)]
                for h in range(NHT):
                    lhs = kt_t[h][:, kt * PT:(kt + 1) * PT]
                    for qc in range(2):
                        nc.tensor.matmul(
                            ps[qc][:], lhs, qt_t[h][:, qc * CH:(qc + 1) * CH],
                            start=(h == 0), stop=(h == NHT - 1),
                        )
                for qc in range(2):
                    nc.scalar.activation(
                        et_slice(kt, qc * CH, CH), ps[qc][:], AF.Exp, scale=SCALE,
                    )

            # ---- V full loads (program-after B so B's waits exclude them) ----
            v_t = []
            for st in range(NKT):
                vtile = pv.tile([PT, H], dt.bfloat16, tag="v")
                v_t.append(vtile)
                nc.sync.dma_start(
                    out=vtile[:], in_=vf_d[st * PT:(st + 1) * PT, :],
                )

            # ---- phase C: attn @ V, denominator, normalize ----
            for qt in range(NQT):
                dn = pp.tile([PT, 1], dt.float32, tag="big", name="dn")
                po = [pp.tile([PT, CH], dt.float32, tag="big", name="psb")
                      for _ in range(2)]
                for kt in range(NKT):
                    lhs = et_slice(kt, qt * PT, PT)
                    for hc in range(2):
                        nc.tensor.matmul(
                            po[hc][:], lhs, v_t[kt][:, hc * CH:(hc + 1) * CH],
                            start=(kt == 0), stop=(kt == NKT - 1),
                        )
                    nc.tensor.matmul(
                        dn[:], lhs, ones_t[:, 0:1],
                        start=(kt == 0), stop=(kt == NKT - 1),
                    )
                rd = prd.tile([PT, 1], dt.float32, tag="rd")
                nc.vector.reciprocal(rd[:], dn[:])
                for hc in range(2):
                    stage = pstage.tile([PT, CH], dt.float32, tag="st", name="stage")
                    nc.vector.tensor_scalar_mul(stage[:], po[hc][:], rd[:])
                    nc.sync.dma_start(
                        out=y[qt * PT:(qt + 1) * PT, hc * CH:(hc + 1) * CH],
                        in_=stage[:],
                    )

    return nc


def _get_nc():
    global _NC
    if _NC is None:
        nc = _build()
        nc.finalize()
        _NC = nc
    return _NC


def kernel(x, Wq, bq, Wk, bk, Wv, bv):
    from concourse.bass_utils import run_bass_kernel_spmd

    nc = _get_nc()

    wq_b = np.ascontiguousarray(Wq.astype(BF16))
    wk_b = np.ascontiguousarray(Wk.astype(BF16))
    wv_b = np.ascontiguousarray(Wv.astype(BF16))
    bq_r = np.ascontiguousarray(bq.reshape(NHT, PT).T.astype(np.float32))
    bk_b = np.ascontiguousarray(np.broadcast_to(bk.astype(BF16), (PT, H)))
    bv_b = np.ascontiguousarray(np.broadcast_to(bv.astype(BF16), (PT, H)))

    in_maps = []
    for c in range(NCORES):
        b, qh = divmod(c, 2)
        xq_c = np.ascontiguousarray(x[b, qh * QH:(qh + 1) * QH, :].T.astype(BF16))
        in_maps.append({
            "xq": xq_c,
            "wq": wq_b, "wk": wk_b, "wv": wv_b,
            "bqr": bq_r, "bkb": bk_b, "bvb": bv_b,
        })

    trace = bool(os.environ.get("BASS_KERNEL_TRACE"))
    kwargs = {}
    if trace:
        _register_ntff_hook()
        kwargs = {"trace": True, "tmpdir": os.environ.get("BASS_KERNEL_TRACE_DIR")}

    res = run_bass_kernel_spmd(nc, in_maps, list(range(NCORES)), **kwargs)
    if trace:
        kernel.last_exec_time_ns = res.exec_time_ns
        kernel.last_results = res

    out = np.empty((B, S, H), np.float32)
    for c in range(NCORES):
        b, qh = divmod(c, 2)
        out[b, qh * QH:(qh + 1) * QH, :] = res.results[c]["y"]
    return out


def _register_ntff_hook():
    """The container's antenv lacks axon_hooks; register it so trace=True
    can capture NTFF profiles through the axon PJRT library."""
    import sys
    import types

    if "antenv.axon_hooks" in sys.modules:
        return
    mod = types.ModuleType("antenv.axon_hooks")
    holder = [None]
    mod.set_axon_ntff_profile_hook = lambda h: holder.__setitem__(0, h)
    mod.get_axon_ntff_profile_hook = lambda: holder[0]
    sys.modules["antenv.axon_hooks"] = mod
    import antenv

    antenv.axon_hooks = mod
    from trn_agent_boot.trn_boot import _ntff_profile_via_ctypes

    mod.set_axon_ntff_profile_hook(_ntff_profile_via_ctypes("/opt/axon/libaxon_pjrt.so"))
